# revision 4
# baseline (speedup 1.0000x reference)
"""ConvLSTMCell Trainium2 kernel (8 NeuronCores, SPMD).

Problem (see reference): xi [4, 256, 16, 64, 64], W [256, 64, 3, 3], b [256]
  t=0:  gates from x0 directly, c0 = sig(i)*tanh(g), h0 = sig(o)*lrelu(c0)
  t>=1: tmp = conv3x3(h, W) + b + x_t;  c = sig(f)*c + sig(i)*tanh(g);
        h = sig(o)*lrelu(c)
Output: h stacked over t -> [4, 64, 16, 64, 64].

Sharding: 8 cores = (batch b, H-half). Each core computes a shrinking
redundant halo (region = its 32 rows + (15-t) extra rows toward the cut) so
there is NO inter-core communication. Bottom-half cores get their rows (and
W's ky axis) flipped host-side so all 8 cores run an identical program.

Per-core layout (channel-major):
  - out-channel permutation [i(0:64) f(64:128)] / [g(0:64) o(64:128)] so all
    elementwise ops are partition-band aligned (no cross-partition DVE ops).
  - conv = 7 matmul passes per 128-out-ch half per row-chunk:
      1 x K=128 identity (adds x_t into PSUM)
      3 x K=128 packed pair (partitions 64:128 = h padded, 0:64 = h shifted
        one col so one matmul covers taps (ky,kx=0)+(ky,kx=1))
      3 x K=64 singles (tap (ky,kx=2) read from the shifted copy)
  - gates: ACT sigmoid/tanh/lrelu with fused per-partition bias; c' = i*g+f*c
    via one DVE mul producing [ig; fc] + a stacked-identity matmul summing the
    bands on PE; h = o*l on GPSIMD. c state + h production all band-aligned.
  - fp32r matmuls (measured relerr ~1.5e-4 on HW).
"""
import numpy as np
from contextlib import ExitStack

import concourse.bacc as bacc
import concourse.tile as tile
from concourse import mybir
from concourse.bass_utils import run_bass_kernel_spmd

F32 = mybir.dt.float32
F32R = mybir.dt.float32r

B, CH4, T, HH, WW = 4, 256, 16, 64, 64
HID = 64
RG = 47            # region rows at t=0 (32 owned + 15 halo)
HP_R, WP = 48, 66  # padded h buffer rows/cols
CH_ROWS = 8        # rows per chunk
GROUP = 3          # chunks per psum group
NFLAT = RG * WW    # 3008

# weight blob columns
PK = [0, 256, 512]          # packed slabs ky=0,1,2  [128, 256]
SG = [768, 1024, 1280]      # single slabs ky=0,1,2  [64, 256] (upper rows 0)
IDC = 1536                  # identity 128           [128, 128]
IST = 1664                  # stacked identity       [128, 128]
WCOLS = 1792


def _emit_timestep_loop(nc, tc, pools, aps, repeats):
    consts, state, xp, hf, gp, op, lp, tp, psc = pools
    x_d, w_d, b_d, out_d = aps

    wb = consts.tile([128, WCOLS], F32R)
    bias = consts.tile([128, 2], F32)
    nc.sync.dma_start(out=wb, in_=w_d)
    nc.sync.dma_start(out=bias, in_=b_d)

    gc = state.tile([128, NFLAT], F32)       # [g ; c]
    hpadA = state.tile([128, HP_R * WP], F32R)
    hpadB = state.tile([128, HP_R * WP], F32R)
    hpads = [hpadA, hpadB]

    SIG = mybir.ActivationFunctionType.Sigmoid
    TANH = mybir.ActivationFunctionType.Tanh
    LRELU = mybir.ActivationFunctionType.Lrelu

    for _ in range(repeats):
        nc.vector.memset(hpadA.bitcast(F32), 0.0)
        nc.vector.memset(hpadB.bitcast(F32), 0.0)
        nc.vector.memset(gc[64:128, :], 0.0)

        for t in range(T):
            Ht = RG - t
            n_flat = Ht * WW
            nch = (Ht + CH_ROWS - 1) // CH_ROWS
            hp_w = hpads[t % 2]
            hp_r = hpads[(t + 1) % 2]
            hp3_w = hp_w.rearrange("p (r w) -> p r w", w=WP)
            hp3_r = hp_r.rearrange("p (r w) -> p r w", w=WP)

            xh = []
            for h in range(2):
                xt = xp.tile([128, NFLAT], F32R)
                x3 = xt.rearrange("p (r w) -> p r w", w=WW)
                nc.sync.dma_start(
                    out=x3[:, 0:Ht, :],
                    in_=x_d[t, 128 * h:128 * h + 128, 0:Ht, :])
                xh.append(xt)

            hfull = hf.tile([128, NFLAT], F32R)
            hf3 = hfull.rearrange("p (r w) -> p r w", w=WW)

            for g0 in range(0, nch, GROUP):
                chunks = list(range(g0, min(g0 + GROUP, nch)))
                ps = None
                if t > 0:
                    ps = [[psc.tile([128, CH_ROWS * WW], F32, tag="ps", name="ps")
                           for _ in chunks] for _ in range(2)]
                    for h in range(2):
                        hw = 128 * h
                        for ky in range(3):
                            for ci, c in enumerate(chunks):
                                rows = min(CH_ROWS, Ht - CH_ROWS * c)
                                n = rows * WW
                                r0 = CH_ROWS * c + ky
                                nc.tensor.matmul(
                                    ps[h][ci][:, 0:n],
                                    wb[:, PK[ky] + hw:PK[ky] + hw + 128],
                                    hp3_r[:, r0:r0 + rows, 0:64],
                                    start=(ky == 0), stop=False)
                        for ky in range(3):
                            for ci, c in enumerate(chunks):
                                rows = min(CH_ROWS, Ht - CH_ROWS * c)
                                n = rows * WW
                                r0 = CH_ROWS * c + ky
                                nc.tensor.matmul(
                                    ps[h][ci][:, 0:n],
                                    wb[0:64, SG[ky] + hw:SG[ky] + hw + 128],
                                    hp3_r[0:64, r0:r0 + rows, 1:65],
                                    start=False, stop=False)
                        for ci, c in enumerate(chunks):
                            rows = min(CH_ROWS, Ht - CH_ROWS * c)
                            n = rows * WW
                            nc.tensor.matmul(
                                ps[h][ci][:, 0:n],
                                wb[:, IDC:IDC + 128],
                                xh[h][:, 512 * c:512 * c + n],
                                start=False, stop=True)

                for ci, c in enumerate(chunks):
                    rows = min(CH_ROWS, Ht - CH_ROWS * c)
                    n = rows * WW
                    win = slice(512 * c, 512 * c + n)
                    if t > 0:
                        s0 = ps[0][ci][:, 0:n]
                        s1 = ps[1][ci][:, 0:n]
                        b0, b1 = bias[:, 0:1], bias[:, 1:2]
                    else:
                        s0 = xh[0][:, win]
                        s1 = xh[1][:, win]
                        b0, b1 = 0.0, 0.0

                    ifs = gp.tile([128, 512], F32)
                    nc.scalar.activation(ifs[:, 0:n], s0, SIG, bias=b0)
                    nc.scalar.activation(
                        gc[0:64, win], s1[0:64, :], TANH,
                        bias=(b1[0:64, :] if t > 0 else 0.0))
                    osb = op.tile([128, 512], F32)
                    nc.scalar.activation(
                        osb[64:128, 0:n], s1[64:128, :], SIG,
                        bias=(b1[64:128, :] if t > 0 else 0.0))

                    tmp = tp.tile([128, 512], F32R)
                    nc.vector.tensor_mul(tmp[:, 0:n], ifs[:, 0:n], gc[:, win])

                    if t > 0:
                        cps = ps[1][ci]
                    else:
                        cps = psc.tile([128, CH_ROWS * WW], F32, tag="ps", name="ps")
                    nc.tensor.matmul(cps[:, 0:n], wb[:, IST:IST + 128],
                                     tmp[:, 0:n], start=True, stop=True)

                    lsb = lp.tile([128, 512], F32)
                    nc.scalar.activation(lsb[64:128, 0:n], cps[64:128, 0:n],
                                         LRELU, alpha=0.01)
                    nc.vector.tensor_copy(gc[64:128, win], cps[64:128, 0:n])
                    nc.gpsimd.tensor_mul(hfull[64:128, win],
                                         osb[64:128, 0:n], lsb[64:128, 0:n])

                    r0 = CH_ROWS * c
                    if t < T - 1:
                        nc.sync.dma_start(
                            out=hp3_w[64:128, 1 + r0:1 + r0 + rows, 1:65],
                            in_=hf3[64:128, r0:r0 + rows, :])
                        nc.sync.dma_start(
                            out=hp3_w[0:64, 1 + r0:1 + r0 + rows, 0:64],
                            in_=hf3[64:128, r0:r0 + rows, :])
                    if r0 < 32:
                        srows = min(rows, 32 - r0)
                        nc.sync.dma_start(
                            out=out_d[:, t, r0:r0 + srows, :],
                            in_=hf3[64:128, r0:r0 + srows, :])


def build_nc(repeats=1):
    nc = bacc.Bacc("TRN2", target_bir_lowering=False, debug=False)
    x_d = nc.dram_tensor("x", [T, CH4, RG, WW], F32R,
                         kind="ExternalInput").ap()
    w_d = nc.dram_tensor("w", [128, WCOLS], F32R, kind="ExternalInput").ap()
    b_d = nc.dram_tensor("bias", [128, 2], F32, kind="ExternalInput").ap()
    out_d = nc.dram_tensor("out", [HID, T, 32, WW], F32R,
                           kind="ExternalOutput").ap()

    with tile.TileContext(nc) as tc, ExitStack() as ctx:
        consts = ctx.enter_context(tc.tile_pool(name="consts", bufs=1))
        state = ctx.enter_context(tc.tile_pool(name="state", bufs=1))
        xp = ctx.enter_context(tc.tile_pool(name="xp", bufs=4))
        hf = ctx.enter_context(tc.tile_pool(name="hf", bufs=2))
        gp = ctx.enter_context(tc.tile_pool(name="gp", bufs=3))
        op = ctx.enter_context(tc.tile_pool(name="op", bufs=3))
        lp = ctx.enter_context(tc.tile_pool(name="lp", bufs=3))
        tp = ctx.enter_context(tc.tile_pool(name="tp", bufs=3))
        psc = ctx.enter_context(
            tc.tile_pool(name="psc", bufs=2 * GROUP, space="PSUM"))
        _emit_timestep_loop(
            nc, tc, (consts, state, xp, hf, gp, op, lp, tp, psc),
            (x_d, w_d, b_d, out_d), repeats)
    nc.compile()
    return nc


def _prep_core_inputs(xi, W, b):
    """Host-side shard prep. Returns list of 8 in_maps."""
    # out-channel permutation: [i f g o]
    perm = np.concatenate([np.arange(0, 128), np.arange(192, 256),
                           np.arange(128, 192)])
    Wp = W[perm]                      # [256, 64, 3, 3]
    bp = b[perm]
    bias_blob = np.stack([bp[0:128], bp[128:256]], axis=1).astype(np.float32)
    bias_blob = np.ascontiguousarray(bias_blob)  # [128, 2]

    def wblob(Wv):
        wb = np.zeros((128, WCOLS), np.float32)
        for ky in range(3):
            wb[0:64, PK[ky]:PK[ky] + 256] = Wv[:, :, ky, 1].T
            wb[64:128, PK[ky]:PK[ky] + 256] = Wv[:, :, ky, 0].T
            wb[0:64, SG[ky]:SG[ky] + 256] = Wv[:, :, ky, 2].T
        wb[:, IDC:IDC + 128] = np.eye(128)
        ist = np.zeros((128, 128), np.float32)
        ist[0:64, 64:128] = np.eye(64)
        ist[64:128, 64:128] = np.eye(64)
        wb[:, IST:IST + 128] = ist
        return wb

    wb_top = wblob(Wp)
    wb_bot = wblob(Wp[:, :, ::-1, :])  # ky flipped for row-flipped cores

    in_maps = []
    for core in range(8):
        bb, half = divmod(core, 2)
        xs = xi[bb][perm]                      # [256, 16, 64, 64]
        if half == 0:
            xs = xs[:, :, 0:RG, :]
        else:
            xs = xs[:, :, ::-1, :][:, :, 0:RG, :]
        xs = np.ascontiguousarray(xs.transpose(1, 0, 2, 3))  # [16,256,47,64]
        in_maps.append({
            "x": xs.astype(np.float32),
            "w": (wb_top if half == 0 else wb_bot),
            "bias": bias_blob,
        })
    return in_maps


_NC_CACHE = {}


def kernel(xi, W, b):
    xi = np.asarray(xi, dtype=np.float32)
    W = np.asarray(W, dtype=np.float32)
    b = np.asarray(b, dtype=np.float32)
    if "nc" not in _NC_CACHE:
        _NC_CACHE["nc"] = build_nc(repeats=1)
    nc = _NC_CACHE["nc"]
    in_maps = _prep_core_inputs(xi, W, b)
    res = run_bass_kernel_spmd(nc, in_maps, list(range(8)), trace=False)
    out = np.empty((B, HID, T, HH, WW), np.float32)
    for core in range(8):
        bb, half = divmod(core, 2)
        o = res.results[core]["out"]          # [64, 16, 32, 64]
        o = np.moveaxis(o, 1, 1)              # [hid, T, 32, W]
        if half == 0:
            out[bb, :, :, 0:32, :] = o
        else:
            out[bb, :, :, 32:64, :] = o[:, :, ::-1, :]
    return out
